# revision 1
# baseline (speedup 1.0000x reference)
"""BatchAllTripletLoss kernel for 8 Trainium2 NeuronCores.

Reference computation:
    pd = pairwise_euclidean(rep)                        # [512, 512]
    tl[a,p,k] = relu(pd[a,p] - pd[a,k] + 5.0) * mask    # [512, 512, 512]
    loss = sum(tl) / (count(tl > eps) + eps)

The mask (p!=a, k!=a, p!=k, label[p]==label[a], label[k]!=label[a])
collapses: label[p]==label[a] and label[k]!=label[a] imply p!=k and k!=a,
so valid triplets are exactly (anchor-positive pairs) x (k with a
different label).  With 64 labels over 512 rows there are only ~4100
(a,p) pairs, so instead of a dense [N,N,N] sweep each core processes its
anchors' pairs as rows of [128-pair, 512-k] tiles:

  per core (64 anchors):
    d[64,512]   = sqrt(relu(aug-matmul))            PE + DVE + ACT
    ym          = d + BIGM*same_label               DVE
    per pair-tile t:
      Gym       = sel_t.T @ ym                      PE one-hot row gather
      x[p]      = sum_k (iota==pidx)*Gym            DVE; = d[a,p] + BIGM
      xp        = x + (margin - BIGM)               DVE
      S_t[p]    = sum_k relu(xp - Gym)              ACT accum
      C_t[p]    = sum_k (Gym < xp)                  DVE accum
    out[1,2*Tp] = ones.T @ [S | C]                  PE partition sum

All matmuls run in float32r (single-pass fp32, ~2^-13 relative rounding;
the one-hot gather then carries that rounding into d).  BIGM = 128 both
masks out same-label k columns (xp <= ~35 << 128 so relu/count give
exactly 0) and carries the bias through the gather; the combined
rounding is ~1e-2 absolute per term, mean-zero, ~1e-4 on the final sums.
rep arrives both row-major (for the row-norm accumulates) and
host-transposed (pure layout permutation) so no PE transposes are
needed.  Anchors are block-sharded 64 per core; the 8 partial
(sum, count) pairs are reduced on the host (the all-reduce of the
sharding hint).  Host-side prep is integer/mask/layout logic only; all
float arithmetic runs on device.
"""

import ml_dtypes
import numpy as np

import concourse.bass as bass
import concourse.tile as tile
from concourse import bacc, mybir
from concourse.bass_utils import run_bass_kernel_spmd
from concourse.vector_clock import ScopedClock


_orig_aeb = bass.Bass.all_engine_barrier


def _skip_const_barrier(self, *, sem_only=False):
    if not getattr(self, "_aeb_skipped_once", False):
        self._aeb_skipped_once = True
        return
    return _orig_aeb(self, sem_only=sem_only)


def _cheap_drain_and_barrier(self, tick_clock, wait_clock):
    """Exit protocol with sequencer-only barriers: the SP drain already
    waits out every engine/DMA tick of the tile clock, so the per-engine
    pipeline drains of the stock double butterfly are redundant here."""
    drain_inst = self.nc.sync.drain()
    wait_clock.add_sem_waits(
        drain_inst.ins, ScopedClock({None: tick_clock.global_clock})
    )
    self.nc.all_engine_barrier(sem_only=True)
    popped = self.nc._tile_sem_poison_stack.pop()
    assert popped is self._sem_poison
    self.nc.clear_and_free_semaphores(list(self.sems.allocated().values()))
    self.nc.all_engine_barrier(sem_only=True)

F32 = mybir.dt.float32
F32R = mybir.dt.float32r
AF = mybir.ActivationFunctionType
OP = mybir.AluOpType

N = 512          # rows
D = 256          # embedding dim
NCORES = 8
A = N // NCORES  # anchors per core
MARGIN = 5.0
EPS = 1e-16
BIG = 1e30       # pad-pair kill value
BIGM = 128.0     # same-label mask / bias carrier (power of two)

_cache = {}


def _build(Tp: int):
    """Build the (uniform, SPMD) per-core Bass program for Tp pair tiles."""
    tile.TileContext._drain_and_barrier = _cheap_drain_and_barrier
    bass.Bass.all_engine_barrier = _skip_const_barrier
    nc = bacc.Bacc(None, target_bir_lowering=False, num_swdge_queues=2)

    rept_d = nc.declare_dram_parameter("rept", [128, 2, N], F32, isOutput=False)
    repa_d = nc.declare_dram_parameter("repa", [A, D], F32, isOutput=False)
    repat_d = nc.declare_dram_parameter("repat", [128, 2, A], F32, isOutput=False)
    bigm_d = nc.declare_dram_parameter("bigm", [A, N], mybir.dt.float8e4, isOutput=False)
    sel_d = nc.declare_dram_parameter("sel", [A, Tp * 128], mybir.dt.float8e4, isOutput=False)
    pm_d = nc.declare_dram_parameter("pm", [128, 2 * Tp], F32, isOutput=False)
    out_d = nc.declare_dram_parameter("out", [1, 2 * Tp], F32, isOutput=True)

    with tile.TileContext(nc) as tc:
        with (
            tc.tile_pool(name="singles", bufs=1) as sg,
            tc.tile_pool(name="scr", bufs=2) as scr,
            tc.tile_pool(name="xs", bufs=3) as xs,
            tc.tile_pool(name="ppf", bufs=1, space="PSUM") as ppf,
            tc.tile_pool(name="ppg", bufs=4, space="PSUM") as ppg,
            tc.tile_pool(name="ppd", bufs=1, space="PSUM") as ppd,
        ):
            iota_f = sg.tile([128, N], F32)
            nc.gpsimd.iota(
                iota_f[:], [[1, N]], channel_multiplier=0,
                allow_small_or_imprecise_dtypes=True,
            )
            ones = sg.tile([128, 1], F32)
            nc.vector.memset(ones[:], 1.0)
            onesr = sg.tile([128, 1], F32R)
            nc.vector.tensor_copy(onesr[:], ones[:])
            ones1 = sg.tile([1, A], F32)
            nc.vector.memset(ones1[:], 1.0)
            ones1r = sg.tile([1, A], F32R)
            nc.vector.tensor_copy(ones1r[:], ones1[:])
            dmy = sg.tile([1, 1], F32)
            nc.scalar.activation(dmy[:], ones[0:1, :], AF.Sqrt, bias=ones[0:1, :])

            # input loads, spread across the two HWDGE queues; rep first
            # (the row-norm chain below is the longest dependency chain)
            rept_s = sg.tile([128, 2, N], F32)     # rept[p, c, j] = rep[j, c*128+p]
            for q in range(4):
                eng = nc.sync if q % 2 == 0 else nc.scalar
                eng.dma_start(
                    rept_s[:, q // 2, (q % 2) * 256:(q % 2) * 256 + 256],
                    rept_d[:, q // 2, (q % 2) * 256:(q % 2) * 256 + 256],
                )
            repat_s = sg.tile([128, 2, A], F32)    # repat[p, c, a] = repa[a, c*128+p]
            nc.gpsimd.dma_start(repat_s[:], repat_d[:])
            repa_s = sg.tile([A, D], F32)
            nc.gpsimd.dma_start(repa_s[:], repa_d[:])
            bigm_s = sg.tile([A, N], mybir.dt.float8e4)
            nc.gpsimd.dma_start(bigm_s[:], bigm_d[:])
            sel_s = sg.tile([A, Tp * 128], mybir.dt.float8e4)
            nc.gpsimd.dma_start(sel_s[:], sel_d[:])
            pm_s = sg.tile([128, 2 * Tp], F32)     # [:, :Tp] pidx, [:, Tp:] margin
            nc.gpsimd.dma_start(pm_s[:], pm_d[:])

            # float32r operand copies (PE consumes pre-rounded data), per
            # chunk so each overlaps the other chunk's DMA
            reptr = sg.tile([128, 2, N], F32R)
            for c in range(2):
                nc.vector.tensor_copy(reptr[:, c, :], rept_s[:, c, :])
            negTa = sg.tile([128, 2, A], F32R)
            nc.vector.tensor_scalar_mul(negTa[:], repat_s[:], -2.0)

            # d2[a, j] = sq_a + sq_j - 2*dot: start the big -2*dot matmuls as
            # soon as the casts land; the sq_j rank-1 terms join the group last
            d2_p = ppd.tile([A, N], F32, tag="d2")
            nc.tensor.matmul(d2_p[:], negTa[:, 0, :], reptr[:, 0, :],
                             start=True, stop=False, skip_group_check=True)
            nc.tensor.matmul(d2_p[:], negTa[:, 1, :], reptr[:, 1, :],
                             start=False, stop=False, skip_group_check=True)

            # sq_row[1, j] = ||rep_j||^2 = ones.T @ (rept * rept)
            sqsq = sg.tile([128, 2, N], F32R)
            for c in range(2):
                nc.vector.tensor_mul(sqsq[:, c, :], rept_s[:, c, :], rept_s[:, c, :])
            sqrow_p = ppf.tile([1, N], F32, tag="fin")
            nc.tensor.matmul(sqrow_p[:], onesr[:], sqsq[:, 0, :], start=True,
                             stop=False, skip_group_check=True)
            nc.tensor.matmul(sqrow_p[:], onesr[:], sqsq[:, 1, :], start=False,
                             stop=True, skip_group_check=True)
            sqrowr = sg.tile([1, N], F32R)
            nc.vector.tensor_copy(sqrowr[:], sqrow_p[:])
            nc.tensor.matmul(d2_p[:], ones1r[:], sqrowr[:], start=False, stop=True,
                             skip_group_check=True)

            # sq_anch[64,1] = ||rep_a||^2
            sqa_scr = scr.tile([A, D], F32, tag="sqa")
            sqanch = sg.tile([A, 1], F32)
            nc.vector.scalar_tensor_tensor(
                out=sqa_scr[:], in0=repa_s[:], scalar=1.0, in1=repa_s[:],
                op0=OP.mult, op1=OP.mult, accum_out=sqanch[:],
            )

            selr = sg.tile([A, Tp * 128], F32R)
            nc.vector.tensor_copy(selr[:], sel_s[:])

            # ym = sqrt(d2 + 0.25) + BIGM*same: the +0.25 keeps the (masked)
            # diagonal's rounding noise out of sqrt's domain; its effect on
            # d_ap - d_ak cancels to ~5e-4
            sqanchb = xs.tile([A, 1], F32, tag="sqb")
            nc.vector.tensor_scalar(sqanchb[:], sqanch[:], 0.25, None, OP.add)
            dtmp = scr.tile([A, N], F32, tag="dtmp")
            nc.scalar.activation(dtmp[:], d2_p[:], AF.Sqrt, bias=sqanchb[:])
            ym = sg.tile([A, N], F32R)
            nc.vector.tensor_add(ym[:], bigm_s[:], dtmp[:])

            # pair tiles
            SC = sg.tile([128, 2 * Tp], F32)
            nc.vector.memset(SC[:], 0.0)
            relbig = sg.tile([128, Tp, N], F32)
            for t in range(Tp):
                gy = ppg.tile([128, N], F32, tag="gy")
                nc.tensor.matmul(gy[:], selr[:, t * 128:(t + 1) * 128], ym[:],
                                 start=True, stop=True)

                stt = scr.tile([128, N], F32, tag="stt")
                xv = xs.tile([128, 1], F32, tag="xv")
                nc.vector.scalar_tensor_tensor(
                    out=stt[:], in0=iota_f[:], scalar=pm_s[:, t:t + 1], in1=gy[:],
                    op0=OP.is_equal, op1=OP.mult, accum_out=xv[:],
                )
                xp = xs.tile([128, 1], F32, tag="xp")
                nc.vector.tensor_scalar(
                    xp[:], xv[:], pm_s[:, Tp + t:Tp + t + 1], None, OP.add
                )

                nc.scalar.activation(
                    relbig[:, t, :], gy[:], AF.Relu, bias=xp[:], scale=-1.0,
                    accum_out=SC[:, t:t + 1],
                )

            # counts: relu output is positive exactly where a triplet is
            # positive, so two wide scans replace five per-tile ones
            h = (Tp + 1) // 2
            nc.vector.tensor_scalar(
                relbig[:, 0:h, :], relbig[:, 0:h, :], 0.0, 0.0, OP.is_gt, OP.add,
                accum_out=SC[:, Tp:Tp + 1],
            )
            if Tp > h:
                nc.vector.tensor_scalar(
                    relbig[:, h:Tp, :], relbig[:, h:Tp, :], 0.0, 0.0,
                    OP.is_gt, OP.add,
                    accum_out=SC[:, Tp + 1:Tp + 2],
                )

            # partition-sum S and C columns -> [1, 2*Tp]
            fin_p = ppf.tile([1, 2 * Tp], F32, tag="fin")
            nc.tensor.matmul(fin_p[:], ones[:], SC[:], start=True, stop=True)
            outsb = sg.tile([1, 2 * Tp], F32)
            nc.vector.tensor_copy(outsb[:], fin_p[:])
            nc.sync.dma_start(out_d[:], outsb[:])

    nc.finalize()
    return nc


def _prep(rep: np.ndarray, labels: np.ndarray):
    """Host-side integer/mask/layout prep: shard anchors, enumerate pairs."""
    rep = np.ascontiguousarray(np.asarray(rep, dtype=np.float32))
    labels = np.asarray(labels)
    same = labels[:, None] == labels[None, :]

    # rep.T packed [128, 2, N]: rept[p, c, j] = rep[j, c*128 + p]
    rept = np.ascontiguousarray(
        rep.T.reshape(2, 128, N).transpose(1, 0, 2)
    )

    pairs = []
    for c in range(NCORES):
        base = c * A
        prs = [
            (j, p)
            for j in range(A)
            for p in np.nonzero(same[base + j])[0]
            if p != base + j
        ]
        pairs.append(prs)
    Tp = max(1, max((len(p) + 127) // 128 for p in pairs))

    in_maps = []
    for c in range(NCORES):
        base = c * A
        repa = rep[base:base + A]
        repat = np.ascontiguousarray(
            repa.T.reshape(2, 128, A).transpose(1, 0, 2)
        )
        bigm = np.where(same[base:base + A], BIGM, 0.0).astype(ml_dtypes.float8_e4m3)
        sel = np.zeros((A, Tp * 128), ml_dtypes.float8_e4m3)
        pm = np.zeros((128, 2 * Tp), np.float32)
        pm[:, Tp:] = -BIG
        for i, (j, p) in enumerate(pairs[c]):
            t, r = divmod(i, 128)
            sel[j, i] = 1.0
            pm[r, t] = p
            pm[r, Tp + t] = MARGIN - BIGM
        in_maps.append({
            "rept": rept,
            "repa": repa,
            "repat": repat,
            "bigm": bigm,
            "sel": sel,
            "pm": pm,
        })
    return Tp, in_maps


def _run(rep, labels, trace=False):
    Tp, in_maps = _prep(rep, labels)
    if Tp not in _cache:
        _cache[Tp] = _build(Tp)
    nc = _cache[Tp]
    res = run_bass_kernel_spmd(nc, in_maps, list(range(NCORES)), trace=trace)
    outs = np.stack([res.results[c]["out"][0] for c in range(NCORES)])  # [8, 2*Tp]
    S = float(outs[:, :Tp].sum())
    C = float(outs[:, Tp:].sum())
    loss = np.float32(S / (C + EPS))
    return np.asarray(loss, dtype=np.float32), res


def kernel(rep, labels):
    loss, _ = _run(rep, labels, trace=False)
    return loss



# revision 12
# speedup vs baseline: 1.1752x; 1.1752x over previous
"""BatchAllTripletLoss kernel for 8 Trainium2 NeuronCores.

Reference computation:
    pd = pairwise_euclidean(rep)                        # [512, 512]
    tl[a,p,k] = relu(pd[a,p] - pd[a,k] + 5.0) * mask    # [512, 512, 512]
    loss = sum(tl) / (count(tl > eps) + eps)

Valid triplets are (anchor-positive pairs) x (k with a different label):
with 64 labels over 512 rows there are ~3930 (a,p) pairs. Anchors are
partitioned into 8 groups of exactly 64, chosen so per-core pair counts
balance to <=512 (4 tiles of 128 pairs). Per core the columns of the
distance matrix are permuted so the core's 64 anchors come first:

  d[64,512]  = sqrt(-2*(dot - sq_a/2 - sq_j/2) + 1)      PE + ACT
  per pair tile t (128 pairs):
    gy       = selaug.T @ [d ; Lk]                       PE (K=128)
    xv[p]    = sum_k (iota==pidx)*gy                     DVE (= d_ap + B)
    xp       = xv + (margin - B)                         DVE (= d_ap + m)
    S_t[p]   = sum_k relu(xp - gy)                       ACT accum
    C_t[p]   = sum_k (relu > 0)                          Pool accum

The same-label mask rides inside the gather matmul: stationary rows
64:128 hold B*onehot(label(anchor)) and the moving tensor's partitions
64:128 hold the label-indicator rows Lk, so gy[k] = d_ak + B*same(a,k)
comes out of PSUM with no separate mask pass.  B = 64 kills masked k in
both relu and count (xp <= ~40 << 64+d_ak) and carries d_ap through
column p.  The +1 inside sqrt keeps the (masked) diagonal's rounding
noise out of sqrt's domain; its effect on d_ap - d_ak cancels to ~1e-4.

All device data is bf16 (inputs are cast host-side; a pure dtype cast).
The 8 cores' per-partition partial sums/counts [128, 2*Tp] are reduced
on the host (the all-reduce of the sharding hint).  Host-side prep is
otherwise integer/mask/layout logic only.

Exit protocol: bass semaphores are allocated from [207,256) — the range
the runtime's end-of-NEFF sweep assigns to the SYNC engine — and the
tile exit emits ONLY a SYNC drain that waits out the full tile clock.
Every other engine's stream ends at its last real instruction, so the
runtime's fixed ~250-semaphore zeroing sweep (~6us, the old exit tail)
overlaps the kernel's own tail instead of serializing after it.  Sync
is the last engine standing, so its sweep range (= all bass sems) is
zeroed only after every consumer has passed.
"""

import ml_dtypes
import numpy as np

import concourse.bass as bass
import concourse.tile as tile
from concourse import bacc, mybir
from concourse.bass_utils import run_bass_kernel_spmd
from concourse.vector_clock import ScopedClock

F32 = mybir.dt.float32
BF16 = mybir.dt.bfloat16
FP16 = mybir.dt.float16
AF = mybir.ActivationFunctionType
OP = mybir.AluOpType

N = 512          # rows
D = 256          # embedding dim
NCORES = 8
A = N // NCORES  # anchors per core
NLAB = 64        # label values
MARGIN = 5.0
EPS = 1e-16
BIGB = 64.0      # same-label mask bias (power of two)
XOFF = MARGIN - BIGB

_orig_aeb = bass.Bass.all_engine_barrier
_orig_sem_range = bass.get_kernel_semaphore_range


def _skip_const_barrier(self, *, sem_only=False):
    # The runtime prologue already barriers all engines before bass code.
    if not getattr(self, "_aeb_skipped_once", False):
        self._aeb_skipped_once = True
        return
    return _orig_aeb(self, sem_only=sem_only)


SAFE_EXIT = True


def _safe_exit(self, tick_clock, wait_clock):
    """Baseline exit: SP drain waits the tile clock, then sem cleanup and
    sequencer-only barriers (proven on hardware)."""
    drain_inst = self.nc.sync.drain()
    wait_clock.add_sem_waits(
        drain_inst.ins, ScopedClock({None: tick_clock.global_clock})
    )
    self.nc.all_engine_barrier(sem_only=True)
    popped = self.nc._tile_sem_poison_stack.pop()
    assert popped is self._sem_poison
    self.nc.clear_and_free_semaphores(list(self.sems.allocated().values()))
    self.nc.all_engine_barrier(sem_only=True)


def _sync_only_exit(self, tick_clock, wait_clock):
    """Exit protocol: a single SYNC drain waiting the full tile clock.

    No all-engine barrier, no semaphore-clear instructions: the runtime's
    end-of-NEFF sweep zeroes every semaphore anyway, and bass sems live
    in SYNC's sweep range (207-255), which runs strictly after this
    drain.  Python-side bookkeeping mirrors clear_and_free_semaphores.
    """
    drain_inst = self.nc.sync.drain()
    wait_clock.add_sem_waits(
        drain_inst.ins, ScopedClock({None: tick_clock.global_clock})
    )
    popped = self.nc._tile_sem_poison_stack.pop()
    assert popped is self._sem_poison
    sem_nums = [s.num for s in self.sems.allocated().values()]
    self.nc._state.prepend_free_semaphores(sem_nums)
    for poison_set in self.nc._tile_sem_poison_stack:
        poison_set.update(sem_nums)

_cache = {}


def _build(Tp: int):
    """Build the (uniform, SPMD) per-core Bass program for Tp pair tiles."""
    tile.TileContext._drain_and_barrier = (
        _safe_exit if SAFE_EXIT else _sync_only_exit)
    bass.Bass.all_engine_barrier = _skip_const_barrier
    if not SAFE_EXIT:
        bass.get_kernel_semaphore_range = lambda: range(207, 256)
    try:
        nc = bacc.Bacc(None, target_bir_lowering=False, num_swdge_queues=2)
    finally:
        bass.get_kernel_semaphore_range = _orig_sem_range

    rept_d = nc.declare_dram_parameter("rept", [128, 2, N], BF16, isOutput=False)
    lk_d = nc.declare_dram_parameter("lk", [NLAB, N], BF16, isOutput=False)
    sel_d = nc.declare_dram_parameter("sel", [128, Tp * 128], BF16, isOutput=False)
    pidx_d = nc.declare_dram_parameter("pidx", [128, Tp], F32, isOutput=False)
    out_d = nc.declare_dram_parameter("out", [128, 2 * Tp], F32, isOutput=True)

    with tile.TileContext(nc) as tc:
        with (
            tc.tile_pool(name="singles", bufs=1) as sg,
            tc.tile_pool(name="xs", bufs=2) as xs,
            tc.tile_pool(name="ppd", bufs=1, space="PSUM") as ppd,
            tc.tile_pool(name="ppf", bufs=1, space="PSUM") as ppf,
            tc.tile_pool(name="ppg", bufs=4, space="PSUM") as ppg,
        ):
            # input loads first: rept chunks on the two HWDGE queues (SP,
            # DVE), everything else on the pool SWDGE queues
            rept_s = sg.tile([128, 2, N], BF16)
            nc.sync.dma_start(rept_s[:, 0, :], rept_d[:, 0, :])
            nc.gpsimd.dma_start(rept_s[:, 1, :], rept_d[:, 1, :])
            sel_s = sg.tile([128, Tp * 128], BF16)
            nc.gpsimd.dma_start(sel_s[:], sel_d[:])
            # M: rows 0:64 = d (written by sqrt), rows 64:128 = Lk
            M = sg.tile([128, N], BF16)
            nc.gpsimd.dma_start(M[A:128, :], lk_d[:])
            pidx_s = sg.tile([128, Tp], F32)
            nc.gpsimd.dma_start(pidx_s[:], pidx_d[:])

            # constants / one-offs that overlap the DMA wait
            iota_f = sg.tile([128, N], F32)
            nc.gpsimd.iota(
                iota_f[:], [[1, N]], channel_multiplier=0,
                allow_small_or_imprecise_dtypes=True,
            )
            onesrow = sg.tile([1, N], FP16)
            nc.vector.memset(onesrow[:], 1.0)
            ones1 = sg.tile([1, A], FP16)
            nc.vector.memset(ones1[:], 1.0)
            onescol = sg.tile([128, 1], FP16)
            nc.vector.memset(onescol[:], 1.0)
            onef = sg.tile([1, 1], F32)
            nc.vector.memset(onef[:], 1.0)
            neghalf = sg.tile([1, 1], F32)
            nc.vector.memset(neghalf[:], -0.5)
            negone = sg.tile([128, 1], F32)
            nc.vector.memset(negone[:], -1.0)
            sqb = sg.tile([A, 1], F32)
            nc.vector.memset(sqb[:], 1.0)
            sqsc = sg.tile([A, 1], F32)
            nc.vector.memset(sqsc[:], -2.0)
            dmy = sg.tile([1, 1], F32)
            nc.scalar.activation(dmy[:], onef[:], AF.Sqrt, bias=onef[:],
                                 scale=onef[:])

            # d2 accumulation group: dot - 0.5*sq_a - 0.5*sq_j (columns
            # 0:64 of the permuted layout ARE the anchors, so sq_a is a
            # slice of the same sq row)
            d2_p = ppd.tile([A, N], F32, tag="d2")
            sqsq = sg.tile([128, 2, N], FP16)
            sqrow_p = ppf.tile([1, N], F32, tag="sqrow")
            sqrowm = sg.tile([1, N], FP16)
            nc.tensor.matmul(d2_p[:], rept_s[:, 0, 0:A], rept_s[:, 0, :],
                             start=True, stop=False, skip_group_check=True)
            nc.vector.tensor_tensor(sqsq[:, 0, :], rept_s[:, 0, :],
                                    rept_s[:, 0, :], OP.mult)
            nc.tensor.matmul(sqrow_p[:], onescol[:], sqsq[:, 0, :],
                             start=True, stop=False, skip_group_check=True)
            nc.tensor.matmul(d2_p[:], rept_s[:, 1, 0:A], rept_s[:, 1, :],
                             start=False, stop=False, skip_group_check=True)
            nc.vector.tensor_tensor(sqsq[:, 1, :], rept_s[:, 1, :],
                                    rept_s[:, 1, :], OP.mult)
            nc.tensor.matmul(sqrow_p[:], onescol[:], sqsq[:, 1, :],
                             start=False, stop=True, skip_group_check=True)
            # -0.5 * sq, split across DVE and ACT (single-partition ops)
            nc.vector.tensor_scalar(sqrowm[0:1, 0:256], sqrow_p[0:1, 0:256],
                                    -0.5, None, OP.mult)
            nc.scalar.activation(sqrowm[0:1, 256:512], sqrow_p[0:1, 256:512],
                                 AF.Copy, scale=-0.5)
            nc.tensor.matmul(d2_p[:], sqrowm[0:1, 0:A], onesrow[:],
                             start=False, stop=False, skip_group_check=True)
            nc.tensor.matmul(d2_p[:], ones1[:], sqrowm[:],
                             start=False, stop=True, skip_group_check=True)

            # d = sqrt(-2*P + 1) into M rows 0:64
            nc.scalar.activation(M[0:A, :], d2_p[:], AF.Sqrt,
                                 bias=sqb[:], scale=-2.0)

            # pair tiles: per tile the baseline-proven stt extracts
            # xv = d_ap + B from PSUM, a small add biases it to
            # xp = d_ap + margin, ACT computes relu(xp - gy) with S accum,
            # and a bf16 count pass over the relu output accumulates C.
            SC = sg.tile([128, 2 * Tp], F32)
            relbig = sg.tile([128, Tp, N], BF16)
            cscr = sg.tile([128, Tp, N], BF16)
            xscr = sg.tile([128, 2, N], F32)
            xv = sg.tile([128, Tp], F32)
            xp = sg.tile([128, Tp], F32)
            gys = []
            for t in range(Tp):
                gy = ppg.tile([128, N], F32, tag="gy", name=f"gy{t}")
                nc.tensor.matmul(gy[:], sel_s[:, t * 128:(t + 1) * 128], M[:],
                                 start=True, stop=True)
                gys.append(gy)

            def extract(t):
                nc.vector.scalar_tensor_tensor(
                    out=xscr[:, t % 2, :], in0=iota_f[:],
                    scalar=pidx_s[:, t:t + 1], in1=gys[t][:],
                    op0=OP.is_equal, op1=OP.mult, accum_out=xv[:, t:t + 1],
                )
                nc.vector.tensor_scalar(xp[:, t:t + 1], xv[:, t:t + 1],
                                        XOFF, None, OP.add)

            def relu(t):
                nc.scalar.activation(
                    relbig[:, t, :], gys[t][:], AF.Relu, bias=xp[:, t:t + 1],
                    scale=-1.0, accum_out=SC[:, t:t + 1],
                )

            def count(t):
                nc.vector.tensor_scalar(
                    cscr[:, t, :], relbig[:, t, :], 0.0, 0.0,
                    OP.is_gt, OP.add, accum_out=SC[:, Tp + t:Tp + t + 1],
                )

            extract(0)
            relu(0)
            for t in range(1, Tp):
                extract(t)
                relu(t)
                count(t - 1)
            count(Tp - 1)

            nc.sync.dma_start(out_d[:], SC[:])

    nc.finalize()
    return nc


def _prep(rep: np.ndarray, labels: np.ndarray):
    """Host-side integer/mask/layout prep: shard anchors, enumerate pairs."""
    rep = np.ascontiguousarray(np.asarray(rep, dtype=np.float32))
    labels = np.asarray(labels).astype(np.int64)
    cnt = np.bincount(labels, minlength=NLAB)
    ppa = cnt[labels] - 1              # pairs per anchor
    rows_of = [np.nonzero(labels == l)[0] for l in range(NLAB)]

    # balance pair counts across 8 groups of exactly 64 anchors
    order = np.argsort(-ppa, kind="stable")
    groups = [[] for _ in range(NCORES)]
    loads = [0] * NCORES
    for a in order:
        cand = min((i for i in range(NCORES) if len(groups[i]) < A),
                   key=lambda j: loads[j])
        groups[cand].append(int(a))
        loads[cand] += int(ppa[a])
    Tp = max(1, (max(loads) + 127) // 128)

    rep_bf = rep.astype(ml_dtypes.bfloat16)
    in_maps = []
    for c in range(NCORES):
        anchors = groups[c]
        inset = np.zeros(N, bool)
        inset[anchors] = True
        perm = np.concatenate([np.array(anchors, np.int64),
                               np.nonzero(~inset)[0]])
        colof = np.empty(N, np.int64)
        colof[perm] = np.arange(N)

        repp = rep_bf[perm]                               # [512, 256]
        rept = np.ascontiguousarray(
            repp.T.reshape(2, 128, N).transpose(1, 0, 2)  # [128, 2, 512]
        )
        lk = (labels[perm][None, :] == np.arange(NLAB)[:, None]).astype(
            ml_dtypes.bfloat16)                           # [64, 512]

        sel = np.zeros((128, Tp * 128), ml_dtypes.bfloat16)
        pidx = np.zeros((128, Tp), np.float32)
        i = 0
        for a, ga in enumerate(anchors):
            for p in rows_of[labels[ga]]:
                if p == ga:
                    continue
                t, r = divmod(i, 128)
                sel[a, i] = 1.0
                sel[A + labels[ga], i] = BIGB
                pidx[r, t] = colof[p]
                i += 1
        in_maps.append({"rept": rept, "lk": lk, "sel": sel, "pidx": pidx})
    return Tp, in_maps


def _run(rep, labels, trace=False):
    Tp, in_maps = _prep(rep, labels)
    if Tp not in _cache:
        _cache[Tp] = _build(Tp)
    nc = _cache[Tp]
    res = run_bass_kernel_spmd(nc, in_maps, list(range(NCORES)), trace=trace)
    outs = np.stack([res.results[c]["out"] for c in range(NCORES)])  # [8,128,2Tp]
    S = float(outs[:, :, :Tp].sum(dtype=np.float64))
    C = float(outs[:, :, Tp:].sum(dtype=np.float64))
    loss = np.float32(S / (C + EPS))
    return np.asarray(loss, dtype=np.float32), res


def kernel(rep, labels):
    loss, _ = _run(rep, labels, trace=False)
    return loss
